# revision 1
# baseline (speedup 1.0000x reference)
"""ConvLSTM (B=4, T=8, C=HID=256, H=W=32, 3x3 SAME convs) on 8 TRN2 NeuronCores.

Sharding: data-parallel over batch (4) x spatial halves of H (2) = 8 cores,
zero inter-core communication. Each core computes its half's rows plus a
shrinking halo margin: at step t it computes 23-t rows; wrong values erode
inward from the un-owned edge at 1 row/step, leaving exactly the owned 16
rows correct after T=8 steps. Upper halves are row-flipped host-side (with
dy-flipped kernels) so all 8 cores run the same SPMD instruction stream.

Compute: conv as 36 PE matmuls per output tile (2 convs x 2 ic-tiles x 9
taps), float32r (fp32 rounded to 11-bit mantissa) at full PE rate, fp32
PSUM accumulation. Gates: sigmoid on ScalarE (bias fused), relu+bias on
VectorE. State update on VectorE. x-conv matmuls are issued before h-conv
matmuls in each chunk so the PE stays busy across the recurrence boundary.
"""
import numpy as np
from contextlib import ExitStack

import concourse.bass as bass
import concourse.tile as tile
from concourse import bacc, mybir
from concourse.bass_utils import run_bass_kernel_spmd

F32R = mybir.dt.float32r
F32 = mybir.dt.float32
AF = mybir.ActivationFunctionType
ALU = mybir.AluOpType

N_CORES = 8
T = 8
ROWS = 26          # h/x buffer rows: p=0 is the y=-1 zero row, p=1..24 = y=0..23
WC = 34            # padded width
PLANE = ROWS * WC  # 884
CROWS = 23         # c buffer rows (max computed rows), 23*32 = 736 per ic-tile
CPL = CROWS * 32

_cache = {}

# tap order: dy=1 row first so the start=True matmul is always full-width
# (dy=0 taps at the top chunk are shrunk by one row — they'd read the
# permanent zero row for output row 1, contributing nothing)
KORD = [3, 4, 5, 0, 1, 2, 6, 7, 8]


def _chunks(t):
    rt = 23 - t
    if rt > 16:
        r1 = (rt + 1) // 2
        return [(1, r1), (1 + r1, rt - r1)]
    return [(1, rt)]


def _build_nc():
    nc = bacc.Bacc("TRN2", target_bir_lowering=False, debug=False,
                   num_devices=N_CORES)
    x_d = nc.dram_tensor("xb", [T, 128, 2 * PLANE], F32R, kind="ExternalInput").ap()
    w_d = nc.dram_tensor("w", [36, 128, 1024], F32R, kind="ExternalInput").ap()
    b_d = nc.dram_tensor("bias", [128, 8], F32, kind="ExternalInput").ap()
    z_d = nc.dram_tensor("hz", [128, 2 * PLANE], F32R, kind="ExternalInput").ap()
    out_d = nc.dram_tensor("hout", [2, 128, 512], F32R, kind="ExternalOutput").ap()

    with tile.TileContext(nc) as tc, ExitStack() as ctx:
        wp = ctx.enter_context(tc.tile_pool(name="wp", bufs=1))
        xp = ctx.enter_context(tc.tile_pool(name="xp", bufs=2))
        hp = ctx.enter_context(tc.tile_pool(name="hp", bufs=1))
        cp = ctx.enter_context(tc.tile_pool(name="cp", bufs=1))
        bp = ctx.enter_context(tc.tile_pool(name="bp", bufs=1))
        gp = ctx.enter_context(tc.tile_pool(name="gp", bufs=10))
        tp = ctx.enter_context(tc.tile_pool(name="tp", bufs=3))
        pp = ctx.enter_context(tc.tile_pool(name="pp", bufs=8, space="PSUM"))

        bt = bp.tile([128, 8], F32, tag="bias")
        nc.sync.dma_start(bt[:], b_d[:])

        ha = hp.tile([128, 2 * PLANE], F32R, tag="ha")
        hb = hp.tile([128, 2 * PLANE], F32R, tag="hb")
        ct = cp.tile([128, 2 * CPL], F32, tag="c")
        nc.vector.memset(ct[:], 0.0)
        hbufs = [ha, hb]

        # x and the h zero-fills ride the gpsimd (SWDGE) queue so they never
        # wait behind the 18.9MB weight stream on the sync (HWDGE) queue.
        # memset can't emit float32r (ISA check) — zero-init h via DMA.
        x0 = xp.tile([128, 2 * PLANE], F32R, tag="x")
        for lo, hi in ((0, 544), (PLANE, PLANE + 544),
                       (544, PLANE), (PLANE + 544, 2 * PLANE)):
            nc.gpsimd.dma_start(x0[:, lo:hi], x_d[0][:, lo:hi])
        nc.gpsimd.dma_start(hb[:], z_d[:])
        nc.gpsimd.dma_start(ha[:], z_d[:])

        # One tile per weight slice so a matmul only waits on the slice it
        # reads. Gates are host-reordered to [i, o, g, f]: t=0 skips the f
        # octiles, so the x-weight slices' i/o/g columns load first and the
        # f columns + all h-weights follow.
        wxs = [wp.tile([128, 768], F32R, tag=f"wx{j}", name=f"wx{j}")
               for j in range(18)]
        wfs = [wp.tile([128, 256], F32R, tag=f"wf{j}", name=f"wf{j}")
               for j in range(18)]
        whs = [wp.tile([128, 1024], F32R, tag=f"wh{j}", name=f"wh{j}")
               for j in range(18)]
        for j in range(18):
            nc.sync.dma_start(wxs[j][:], w_d[j][:, :768])
        for j in range(18):
            nc.sync.dma_start(wfs[j][:], w_d[j][:, 768:])
        for j in range(18):
            nc.sync.dma_start(whs[j][:], w_d[18 + j])

        def wslice(j, o):
            if j < 18:
                if o < 6:
                    return wxs[j][:, o * 128:(o + 1) * 128]
                return wfs[j][:, (o - 6) * 128:(o - 5) * 128]
            return whs[j - 18][:, o * 128:(o + 1) * 128]

        for t in range(T):
            h_in, h_out = hbufs[t % 2], hbufs[(t + 1) % 2]
            if t == 0:
                xt = x0
            else:
                xt = xp.tile([128, 2 * PLANE], F32R, tag="x")
                nc.gpsimd.dma_start(xt[:], x_d[t])
            xv = xt[:].rearrange("p (i r c) -> p i r c", i=2, r=ROWS, c=WC)
            hv = h_in[:].rearrange("p (i r c) -> p i r c", i=2, r=ROWS, c=WC)
            hov = h_out[:].rearrange("p (i r c) -> p i r c", i=2, r=ROWS, c=WC)

            # t=0: h_0 == 0, so skip all h-conv matmuls; f-gate is unused
            # (f*c_0 == 0), so skip its two octiles entirely.
            # octile order (host-reordered): 0,1=i  2,3=o  4,5=g  6,7=f
            octs = [0, 1, 2, 3, 4, 5] if t == 0 else list(range(8))
            for (q, r) in _chunks(t):
                n = r * 32
                ps_tiles = {}
                # x-conv half first: independent of the recurrence, keeps the
                # PE busy while the previous step's state update drains.
                # At t=0 the weight slices are still streaming in from HBM,
                # so iterate j-major to consume them in arrival order.
                def emit_mm(ps, src, j, o, it, k, start, stop):
                    dy, dx = k // 3, k % 3
                    if q == 1 and dy == 0:
                        # top chunk: dy=0 tap of output row 1 reads the
                        # permanent zero row -> drop that row from the MM
                        nc.tensor.matmul(
                            ps[:, 32:], wslice(j, o),
                            src[:, it, 1: r, dx: dx + 32],
                            start=start, stop=stop, skip_group_check=True)
                    else:
                        nc.tensor.matmul(
                            ps[:], wslice(j, o),
                            src[:, it, q + dy - 1: q + dy - 1 + r,
                                dx: dx + 32],
                            start=start, stop=stop, skip_group_check=True)

                if t == 0:
                    for o in octs:
                        ps_tiles[o] = pp.tile([128, n], F32, tag="ps",
                                              name=f"ps{o}")
                    for it in range(2):
                        for k in KORD:
                            j = it * 9 + k
                            for o in octs:
                                emit_mm(ps_tiles[o], xv, j, o, it, k,
                                        start=(it == 0 and k == KORD[0]),
                                        stop=(it == 1 and k == KORD[-1]))
                else:
                    for o in octs:
                        ps = pp.tile([128, n], F32, tag="ps")
                        ps_tiles[o] = ps
                        for it in range(2):
                            for k in KORD:
                                emit_mm(ps, xv, it * 9 + k, o, it, k,
                                        start=(it == 0 and k == KORD[0]),
                                        stop=False)
                if t > 0:
                    for o in octs:
                        ps = ps_tiles[o]
                        for it in range(2):
                            for k in KORD:
                                emit_mm(ps, hv, 18 + it * 9 + k, o, it, k,
                                        start=False,
                                        stop=(it == 1 and k == KORD[-1]))
                gts = {}
                for o in octs:
                    gt = gp.tile([128, n], F32, tag="g")
                    gts[o] = gt
                    if o < 4 or o >= 6:  # i, o, f -> sigmoid; g -> relu
                        nc.scalar.activation(gt[:], ps_tiles[o][:], AF.Sigmoid,
                                             bias=bt[:, o:o + 1])
                    else:
                        nc.vector.tensor_scalar(gt[:], ps_tiles[o][:],
                                                bt[:, o:o + 1], 0.0,
                                                ALU.add, ALU.max)
                for hi in range(2):
                    gi, go, gg = gts[0 + hi], gts[2 + hi], gts[4 + hi]
                    c0 = hi * CPL + (q - 1) * 32
                    cs = ct[:, c0: c0 + n]
                    if t == 0:
                        nc.vector.tensor_mul(cs, gi[:], gg[:])
                    else:
                        gf = gts[6 + hi]
                        nc.vector.tensor_mul(gg[:], gi[:], gg[:])
                        nc.vector.tensor_mul(cs, gf[:], cs)
                        nc.vector.tensor_add(cs, cs, gg[:])
                    cr = tp.tile([128, n], F32, tag="cr")
                    nc.vector.tensor_scalar_max(cr[:], cs, 0.0)
                    nc.vector.tensor_mul(hov[:, hi, q: q + r, 1: 33], go[:], cr[:])

        hf = hbufs[T % 2][:].rearrange("p (i r c) -> p i r c", i=2, r=ROWS, c=WC)
        for it in range(2):
            nc.sync.dma_start(out_d[it], hf[:, it, 1: 17, 1: 33])

    nc.compile()
    return nc


def _round_f32r(a):
    b = np.ascontiguousarray(a, dtype=np.float32).view(np.uint32)
    b = (b + np.uint32(0x7FF) + ((b >> np.uint32(12)) & np.uint32(1))) \
        & np.uint32(0xFFFFF000)
    return b.view(np.float32)


GATE_PERM = [0, 2, 3, 1]  # reorder [i, f, o, g] -> [i, o, g, f]


def _prep_weights(wx, wh, flip):
    ws = np.stack([np.asarray(wx), np.asarray(wh)])  # [2, 1024, 256, 3, 3]
    if flip:
        ws = ws[:, :, :, ::-1, :]
    # [cv, gate, ht, ch, it, ic, dy, dx] -> [cv, it, dy, dx, ic, gate, ht, ch]
    ws = ws.reshape(2, 4, 2, 128, 2, 128, 3, 3)[:, GATE_PERM]
    ws = ws.transpose(0, 4, 6, 7, 5, 1, 2, 3)
    return _round_f32r(ws.reshape(36, 128, 1024))


def _prep_x(xb, flip):
    # xb: [T, 256, 32, 32] for one batch element -> [T, 128, 2*PLANE]
    xc = np.asarray(xb)
    if flip:
        xc = xc[:, :, ::-1, :]
    buf = np.zeros((T, 2, 128, ROWS, WC), dtype=np.float32)
    for it in range(2):
        buf[:, it, :, 1:25, 1:33] = xc[:, it * 128:(it + 1) * 128, 0:24, :]
    buf = buf.reshape(T, 2, 128, PLANE).transpose(0, 2, 1, 3)
    return _round_f32r(np.ascontiguousarray(buf).reshape(T, 128, 2 * PLANE))


def kernel(x, wx, wh, bh):
    x = np.asarray(x, dtype=np.float32)
    B = x.shape[0]
    bias = np.ascontiguousarray(
        np.asarray(bh, dtype=np.float32).reshape(4, 2, 128)[GATE_PERM]
        .transpose(2, 0, 1).reshape(128, 8))

    w_lo = _prep_weights(wx, wh, flip=False)
    w_hi = _prep_weights(wx, wh, flip=True)

    in_maps = []
    for c in range(N_CORES):
        b, half = c // 2, c % 2
        in_maps.append({
            "xb": _prep_x(x[b], flip=bool(half)),
            "w": w_hi if half else w_lo,
            "bias": bias,
            "hz": np.zeros((128, 2 * PLANE), dtype=np.float32),
        })

    if "nc" not in _cache:
        _cache["nc"] = _build_nc()
    nc = _cache["nc"]

    res = run_bass_kernel_spmd(nc, in_maps, core_ids=list(range(N_CORES)))
    _cache["last_results"] = res

    out = np.zeros((B, 256, 32, 32), dtype=np.float32)
    for c in range(N_CORES):
        b, half = c // 2, c % 2
        h = res.results[c]["hout"].reshape(2, 128, 16, 32)
        h = np.concatenate([h[0], h[1]], axis=0)  # [256, 16, 32]
        if half:
            out[b, :, 16:32, :] = h[:, ::-1, :]
        else:
            out[b, :, 0:16, :] = h
    return out



# revision 2
# speedup vs baseline: 1.6254x; 1.6254x over previous
"""ConvLSTM (B=4, T=8, C=HID=256, H=W=32, 3x3 SAME convs) on 8 TRN2 NeuronCores.

Sharding: data-parallel over batch (4) x spatial halves of H (2) = 8 cores,
zero inter-core communication. Each core computes its half's rows plus a
shrinking halo margin: at step t it computes 23-t rows; wrong values erode
inward from the un-owned edge at 1 row/step, leaving exactly the owned 16
rows correct after T=8 steps. Upper halves are row-flipped host-side (with
dy-flipped kernels) so all 8 cores run the same SPMD instruction stream.

Precision schedule: steps 0..4 run both convs in fp8(e4m3) DoubleRow mode
(K=256 per matmul: the two ic-tiles of a tap are the pair -> 9 matmuls per
conv per octile at 2x PE rate), steps 5..7 run in bf16 (full PE rate, ~2e-3
quantization - negligible). fp8 errors injected at early steps decay through
the forget gate; simulated end-to-end rel err ~1.5e-2 vs the 2e-2 budget.
Scales keep everything inside e4m3's +-240 range: x*16, wx*512, h*8,
wh*1024, so both conv products carry 8192 and one activation scale undoes
it. Gates: ScalarE activation (sigmoid / relu, bias+descale fused); state
update on VectorE writes the next step's h directly in the next step's conv
input dtype (fp8 scaled by 8, bf16, or f32 at the last step).
"""
import numpy as np
import ml_dtypes
from contextlib import ExitStack

import concourse.bass as bass
import concourse.tile as tile
from concourse import bacc, mybir
from concourse.bass_utils import run_bass_kernel_spmd

F8 = mybir.dt.float8e4
BF16 = mybir.dt.bfloat16
F32 = mybir.dt.float32
AF = mybir.ActivationFunctionType
ALU = mybir.AluOpType
DR = mybir.MatmulPerfMode.DoubleRow

N_CORES = 8
T = 8
NF8 = 5            # steps 0..NF8-1 run in fp8
ROWS = 26          # h/x buffer rows: p=0 is the y=-1 zero row, p=1..24 = y=0..23
WC = 34            # padded width
PLANE = ROWS * WC  # 884
CROWS = 23         # c buffer rows (max computed rows), 23*32 = 736 per ic-tile
CPL = CROWS * 32

XS, WXS = 16.0, 512.0     # fp8 scales: x*16, wx*512
HS, WHS = 8.0, 1024.0     # h*8, wh*1024  (same product 8192)
DESCALE = 1.0 / 8192.0

_cache = {}

# tap order: dy=1 row first so the start=True matmul is always full-width
# (dy=0 taps at the top chunk are shrunk by one row — they'd read the
# permanent zero row for output row 1, contributing nothing)
KORD = [3, 4, 5, 0, 1, 2, 6, 7, 8]


def _chunks(t):
    rt = 23 - t
    if rt > 16:
        r1 = (rt + 1) // 2
        return [(1, r1), (1 + r1, rt - r1)]
    return [(1, rt)]


def _build_nc():
    nc = bacc.Bacc("TRN2", target_bir_lowering=False, debug=False,
                   num_devices=N_CORES)
    x8_d = nc.dram_tensor("x8", [NF8, 128, 2 * PLANE], F8, kind="ExternalInput").ap()
    x16_d = nc.dram_tensor("x16", [T - NF8, 128, 2 * PLANE], BF16, kind="ExternalInput").ap()
    w8_d = nc.dram_tensor("w8", [18, 128, 2, 1024], F8, kind="ExternalInput").ap()
    w16_d = nc.dram_tensor("w16", [36, 128, 1024], BF16, kind="ExternalInput").ap()
    b_d = nc.dram_tensor("bias", [128, 8], F32, kind="ExternalInput").ap()
    out_d = nc.dram_tensor("hout", [2, 128, 512], F32, kind="ExternalOutput").ap()

    with tile.TileContext(nc) as tc, ExitStack() as ctx:
        wp = ctx.enter_context(tc.tile_pool(name="wp", bufs=1))
        xp8 = ctx.enter_context(tc.tile_pool(name="xp8", bufs=2))
        xp16 = ctx.enter_context(tc.tile_pool(name="xp16", bufs=2))
        hp = ctx.enter_context(tc.tile_pool(name="hp", bufs=1))
        cp = ctx.enter_context(tc.tile_pool(name="cp", bufs=1))
        bp = ctx.enter_context(tc.tile_pool(name="bp", bufs=1))
        gp = ctx.enter_context(tc.tile_pool(name="gp", bufs=10))
        tp = ctx.enter_context(tc.tile_pool(name="tp", bufs=3))
        pp = ctx.enter_context(tc.tile_pool(name="pp", bufs=8, space="PSUM"))

        bt = bp.tile([128, 8], F32, tag="bias")
        nc.sync.dma_start(bt[:], b_d[:])

        h8a = hp.tile([128, 2 * PLANE], F8, tag="h8a")
        h8b = hp.tile([128, 2 * PLANE], F8, tag="h8b")
        h16a = hp.tile([128, 2 * PLANE], BF16, tag="h16a")
        h16b = hp.tile([128, 2 * PLANE], BF16, tag="h16b")
        hf = hp.tile([128, 1024], F32, tag="hf")
        ct = cp.tile([128, 2 * CPL], F32, tag="c")
        nc.vector.memset(ct[:], 0.0)
        nc.vector.memset(h8a[:], 0.0)
        nc.vector.memset(h8b[:], 0.0)
        nc.vector.memset(h16a[:], 0.0)
        nc.vector.memset(h16b[:], 0.0)

        # x rides the gpsimd (SWDGE) queue so it never waits behind the
        # weight stream on the sync (HWDGE) queue.
        x80 = xp8.tile([128, 2 * PLANE], F8, tag="x8")
        nc.gpsimd.dma_start(x80[:], x8_d[0])

        # fp8 weight tiles, one per tap. x-conv i/o/g columns first (t=0
        # skips the f octiles and all h-convs), issued in KORD consumption
        # order so t=0's matmuls can start as slices arrive.
        w8x = [wp.tile([128, 2, 768], F8, tag=f"w8x{j}", name=f"w8x{j}")
               for j in range(9)]
        w8xf = [wp.tile([128, 2, 256], F8, tag=f"w8xf{j}", name=f"w8xf{j}")
                for j in range(9)]
        w8h = [wp.tile([128, 2, 1024], F8, tag=f"w8h{j}", name=f"w8h{j}")
               for j in range(9)]
        for j in KORD:
            nc.sync.dma_start(w8x[j][:], w8_d[j][:, :, :768])
        for j in KORD:
            nc.sync.dma_start(w8h[j][:], w8_d[9 + j])
        for j in KORD:
            nc.sync.dma_start(w8xf[j][:], w8_d[j][:, :, 768:])
        # bf16 weights (needed from step NF8 on; stream after all fp8)
        w16 = [wp.tile([128, 1024], BF16, tag=f"w16_{j}", name=f"w16_{j}")
               for j in range(36)]
        for j in range(36):
            nc.sync.dma_start(w16[j][:], w16_d[j])

        def wslice8(cv, j, o):
            if cv == 0:
                if o < 6:
                    return w8x[j][:, :, o * 128:(o + 1) * 128]
                return w8xf[j][:, :, (o - 6) * 128:(o - 5) * 128]
            return w8h[j][:, :, o * 128:(o + 1) * 128]

        def wslice16(cv, j, o):
            return w16[cv * 18 + j][:, o * 128:(o + 1) * 128]

        hbufs8 = [h8a, h8b]
        hbufs16 = [h16a, h16b]

        for t in range(T):
            fp8 = t < NF8
            if t == 0:
                xt = x80
            elif fp8:
                xt = xp8.tile([128, 2 * PLANE], F8, tag="x8")
                nc.gpsimd.dma_start(xt[:], x8_d[t])
            else:
                xt = xp16.tile([128, 2 * PLANE], BF16, tag="x16")
                nc.gpsimd.dma_start(xt[:], x16_d[t - NF8])
            h_in = hbufs8[t % 2] if fp8 else hbufs16[t % 2]
            # h_out dtype is the NEXT step's conv input dtype
            if t < NF8 - 1:
                h_out = hbufs8[(t + 1) % 2]
            elif t < T - 1:
                h_out = hbufs16[(t + 1) % 2]
            else:
                h_out = None  # final step writes hf (f32)
            xv = xt[:].rearrange("p (i r c) -> p i r c", i=2, r=ROWS, c=WC)
            hv = h_in[:].rearrange("p (i r c) -> p i r c", i=2, r=ROWS, c=WC)
            hov = (h_out[:].rearrange("p (i r c) -> p i r c", i=2, r=ROWS, c=WC)
                   if h_out is not None else
                   hf[:].rearrange("p (i r c) -> p i r c", i=2, r=16, c=32))

            # t=0: h_0 == 0, so skip all h-conv matmuls; f-gate is unused
            # (f*c_0 == 0), so skip its two octiles entirely.
            # octile order (host-reordered): 0,1=i  2,3=o  4,5=g  6,7=f
            octs = [0, 1, 2, 3, 4, 5] if t == 0 else list(range(8))
            for (q, r) in _chunks(t):
                n = r * 32
                ps_tiles = {}

                def emit8(ps, src, j, o, cv, k, start, stop):
                    dy, dx = k // 3, k % 3
                    if q == 1 and dy == 0:
                        # top chunk: dy=0 tap of output row 1 reads the
                        # permanent zero row -> drop that row from the MM
                        nc.tensor.matmul(
                            ps[:, 32:], wslice8(cv, j, o),
                            src[:, :, 1: r, dx: dx + 32],
                            start=start, stop=stop, perf_mode=DR,
                            skip_group_check=True)
                    else:
                        nc.tensor.matmul(
                            ps[:], wslice8(cv, j, o),
                            src[:, :, q + dy - 1: q + dy - 1 + r, dx: dx + 32],
                            start=start, stop=stop, perf_mode=DR,
                            skip_group_check=True)

                def emit16(ps, src, j, o, cv, it, k, start, stop):
                    dy, dx = k // 3, k % 3
                    if q == 1 and dy == 0:
                        nc.tensor.matmul(
                            ps[:, 32:], wslice16(cv, it * 9 + k, o),
                            src[:, it, 1: r, dx: dx + 32],
                            start=start, stop=stop, skip_group_check=True)
                    else:
                        nc.tensor.matmul(
                            ps[:], wslice16(cv, it * 9 + k, o),
                            src[:, it, q + dy - 1: q + dy - 1 + r,
                                dx: dx + 32],
                            start=start, stop=stop, skip_group_check=True)

                # x-conv half first: independent of the recurrence, keeps the
                # PE busy while the previous step's state update drains.
                # At t=0 the weight slices are still streaming in from HBM,
                # so iterate tap-major to consume them in arrival order.
                if t == 0:
                    for o in octs:
                        ps_tiles[o] = pp.tile([128, n], F32, tag="ps",
                                              name=f"ps{o}")
                    for k in KORD:
                        for o in octs:
                            emit8(ps_tiles[o], xv, k, o, 0, k,
                                  start=(k == KORD[0]), stop=(k == KORD[-1]))
                elif fp8:
                    for o in octs:
                        ps = pp.tile([128, n], F32, tag="ps")
                        ps_tiles[o] = ps
                        for k in KORD:
                            emit8(ps, xv, k, o, 0, k,
                                  start=(k == KORD[0]), stop=False)
                    for o in octs:
                        ps = ps_tiles[o]
                        for k in KORD:
                            emit8(ps, hv, k, o, 1, k,
                                  start=False, stop=(k == KORD[-1]))
                else:
                    for o in octs:
                        ps = pp.tile([128, n], F32, tag="ps")
                        ps_tiles[o] = ps
                        for it in range(2):
                            for k in KORD:
                                emit16(ps, xv, it * 9 + k, o, 0, it, k,
                                       start=(it == 0 and k == KORD[0]),
                                       stop=False)
                    for o in octs:
                        ps = ps_tiles[o]
                        for it in range(2):
                            for k in KORD:
                                emit16(ps, hv, it * 9 + k, o, 1, it, k,
                                       start=False,
                                       stop=(it == 1 and k == KORD[-1]))

                scale = DESCALE if fp8 else 1.0
                gts = {}
                for o in octs:
                    gt = gp.tile([128, n], F32, tag="g")
                    gts[o] = gt
                    if o < 4 or o >= 6:  # i, o, f -> sigmoid; g -> relu
                        nc.scalar.activation(gt[:], ps_tiles[o][:], AF.Sigmoid,
                                             bias=bt[:, o:o + 1], scale=scale)
                    elif fp8:
                        nc.scalar.activation(gt[:], ps_tiles[o][:], AF.Relu,
                                             bias=bt[:, o:o + 1], scale=scale)
                    else:
                        nc.vector.tensor_scalar(gt[:], ps_tiles[o][:],
                                                bt[:, o:o + 1], 0.0,
                                                ALU.add, ALU.max)
                for hi in range(2):
                    gi, go, gg = gts[0 + hi], gts[2 + hi], gts[4 + hi]
                    c0 = hi * CPL + (q - 1) * 32
                    cs = ct[:, c0: c0 + n]
                    if t == 0:
                        nc.vector.tensor_mul(cs, gi[:], gg[:])
                    else:
                        gf = gts[6 + hi]
                        nc.vector.tensor_mul(gg[:], gi[:], gg[:])
                        nc.vector.tensor_mul(cs, gf[:], cs)
                        nc.vector.tensor_add(cs, cs, gg[:])
                    cr = tp.tile([128, n], F32, tag="cr")
                    if t < NF8 - 1:
                        # next step consumes h in fp8 scaled by HS
                        nc.vector.tensor_scalar(cr[:], cs, 0.0, HS,
                                                ALU.max, ALU.mult)
                    else:
                        nc.vector.tensor_scalar_max(cr[:], cs, 0.0)
                    if t == T - 1:
                        nc.vector.tensor_mul(hov[:, hi, :, :], go[:], cr[:])
                    else:
                        nc.vector.tensor_mul(hov[:, hi, q: q + r, 1: 33],
                                             go[:], cr[:])

        for it in range(2):
            nc.sync.dma_start(out_d[it], hf[:].rearrange(
                "p (i r c) -> p i r c", i=2, r=16, c=32)[:, it, :, :])

    nc.compile()
    return nc


def _round_f32r(a):
    b = np.ascontiguousarray(a, dtype=np.float32).view(np.uint32)
    b = (b + np.uint32(0x7FF) + ((b >> np.uint32(12)) & np.uint32(1))) \
        & np.uint32(0xFFFFF000)
    return b.view(np.float32)


def _f8(a, scale):
    return np.clip(np.asarray(a, dtype=np.float32) * scale,
                   -240.0, 240.0).astype(ml_dtypes.float8_e4m3)


GATE_PERM = [0, 2, 3, 1]  # reorder [i, f, o, g] -> [i, o, g, f]


def _prep_weights(wx, wh, flip):
    ws = np.stack([np.asarray(wx), np.asarray(wh)])  # [2, 1024, 256, 3, 3]
    if flip:
        ws = ws[:, :, :, ::-1, :]
    # [cv, gate, ht, ch, it, ic, dy, dx]
    ws = ws.reshape(2, 4, 2, 128, 2, 128, 3, 3)[:, GATE_PERM]
    # -> [cv, dy, dx, ic, it, gate, ht, ch] for fp8 pair layout
    w8 = ws.transpose(0, 6, 7, 5, 4, 1, 2, 3).reshape(2, 9, 128, 2, 1024)
    w8 = np.concatenate([_f8(w8[0], WXS), _f8(w8[1], WHS)])  # [18,128,2,1024]
    # -> [cv, it, dy, dx, ic, gate, ht, ch] for bf16 layout
    w16 = ws.transpose(0, 4, 6, 7, 5, 1, 2, 3).reshape(36, 128, 1024)
    w16 = w16.astype(ml_dtypes.bfloat16)
    return w8, w16


def _prep_x(xb, flip):
    # xb: [T, 256, 32, 32] for one batch element
    xc = np.asarray(xb, dtype=np.float32)
    if flip:
        xc = xc[:, :, ::-1, :]
    buf = np.zeros((T, 2, 128, ROWS, WC), dtype=np.float32)
    for it in range(2):
        buf[:, it, :, 1:25, 1:33] = xc[:, it * 128:(it + 1) * 128, 0:24, :]
    buf = buf.reshape(T, 2, 128, PLANE).transpose(0, 2, 1, 3)
    buf = np.ascontiguousarray(buf).reshape(T, 128, 2 * PLANE)
    return _f8(buf[:NF8], XS), buf[NF8:].astype(ml_dtypes.bfloat16)


def kernel(x, wx, wh, bh):
    x = np.asarray(x, dtype=np.float32)
    B = x.shape[0]
    bias = np.ascontiguousarray(
        np.asarray(bh, dtype=np.float32).reshape(4, 2, 128)[GATE_PERM]
        .transpose(2, 0, 1).reshape(128, 8))

    w8_lo, w16_lo = _prep_weights(wx, wh, flip=False)
    w8_hi, w16_hi = _prep_weights(wx, wh, flip=True)

    in_maps = []
    for c in range(N_CORES):
        b, half = c // 2, c % 2
        x8, x16 = _prep_x(x[b], flip=bool(half))
        in_maps.append({
            "x8": x8,
            "x16": x16,
            "w8": w8_hi if half else w8_lo,
            "w16": w16_hi if half else w16_lo,
            "bias": bias,
        })

    if "nc" not in _cache:
        _cache["nc"] = _build_nc()
    nc = _cache["nc"]

    res = run_bass_kernel_spmd(nc, in_maps, core_ids=list(range(N_CORES)))
    _cache["last_results"] = res

    out = np.zeros((B, 256, 32, 32), dtype=np.float32)
    for c in range(N_CORES):
        b, half = c // 2, c % 2
        h = res.results[c]["hout"].reshape(2, 128, 16, 32)
        h = np.concatenate([h[0], h[1]], axis=0)  # [256, 16, 32]
        if half:
            out[b, :, 16:32, :] = h[:, ::-1, :]
        else:
            out[b, :, 0:16, :] = h
    return out


# revision 6
# speedup vs baseline: 1.8357x; 1.1294x over previous
"""ConvLSTM (B=4, T=8, C=HID=256, H=W=32, 3x3 SAME convs) on 8 TRN2 NeuronCores.

Sharding: data-parallel over batch (4) x spatial halves of H (2) = 8 cores,
zero inter-core communication. Each core computes its half's rows plus a
shrinking halo margin: at step t it computes 23-t rows; wrong values erode
inward from the un-owned edge at 1 row/step, leaving exactly the owned 16
rows correct after T=8 steps. Upper halves are row-flipped host-side (with
dy-flipped kernels) so all 8 cores run the same SPMD instruction stream.

Compute: 1D Winograd F(2,3) along W for both convs at every step: per
output-channel octile, 3(dy) x 4(pos) matmuls over K=ic accumulate four
position planes M_p, and VectorE applies A^T ([m0+m1+m2, m1-m2-m3]) to
produce the 16 even/odd column pairs - 1.5x fewer PE columns than direct
conv, and every step fits a single 512-col PSUM chunk.

Precision: steps 0..3 quantize V (data transform) and U (weight transform)
to fp8(e4m3) and run DoubleRow matmuls (ic-pair, K=256, 2x PE rate); steps
4..7 run bf16. fp8 errors injected at early steps decay through the forget
gate; simulated end-to-end rel err ~1.5e-2 vs the 2e-2 budget. Scales keep
e4m3 in range (x*16/wx*512, h*8/wh*1024 - both products 8192, undone by the
activation scale). The h data transform runs on VectorE (fp8 steps) or
GpSimd (bf16 steps); the x transform is precomputed host-side.
"""
import numpy as np
import ml_dtypes
from contextlib import ExitStack

import concourse.bass as bass
import concourse.tile as tile
from concourse import bacc, mybir
from concourse.bass_utils import run_bass_kernel_spmd

F8 = mybir.dt.float8e4
BF16 = mybir.dt.bfloat16
F32 = mybir.dt.float32
AF = mybir.ActivationFunctionType
ALU = mybir.AluOpType
DR = mybir.MatmulPerfMode.DoubleRow

N_CORES = 8
T = 8
NF8 = 4            # steps 0..NF8-1 run fp8 Winograd; the rest bf16 Winograd
ROWS = 26          # h plane rows: p=0 is the y=-1 zero row, p=1..24 = y=0..23
WC = 34            # padded width
PLANE = ROWS * WC  # 884
CROWS = 23
CPL = CROWS * 32
VR8 = 25           # V rows, fp8 steps (t=0 reads dy..dy+22, dy<=2)
VR16 = 21          # V rows, bf16 steps (t=4 reads dy..dy+18)

XS, WXS = 16.0, 512.0
HS, WHS = 8.0, 1024.0
DESCALE = 1.0 / 8192.0

_cache = {}


def _build_nc():
    nc = bacc.Bacc("TRN2", target_bir_lowering=False, debug=False,
                   num_devices=N_CORES)
    vx8_d = nc.dram_tensor("vx8", [NF8, 128, 2 * 4 * VR8 * 16], F8,
                           kind="ExternalInput").ap()
    vx16_d = nc.dram_tensor("vx16", [T - NF8, 128, 2 * 4 * VR16 * 16], BF16,
                            kind="ExternalInput").ap()
    u8_d = nc.dram_tensor("u8", [24, 128, 2, 1024], F8, kind="ExternalInput").ap()
    u16_d = nc.dram_tensor("u16", [48, 128, 1024], BF16, kind="ExternalInput").ap()
    b_d = nc.dram_tensor("bias", [128, 8], F32, kind="ExternalInput").ap()
    out_d = nc.dram_tensor("hout", [2, 128, 512], F32, kind="ExternalOutput").ap()

    with tile.TileContext(nc) as tc, ExitStack() as ctx:
        wp = ctx.enter_context(tc.tile_pool(name="wp", bufs=1))
        vxp8 = ctx.enter_context(tc.tile_pool(name="vxp8", bufs=2))
        vxp16 = ctx.enter_context(tc.tile_pool(name="vxp16", bufs=2))
        vhp8 = ctx.enter_context(tc.tile_pool(name="vhp8", bufs=1))
        vhp16 = ctx.enter_context(tc.tile_pool(name="vhp16", bufs=1))
        hp = ctx.enter_context(tc.tile_pool(name="hp", bufs=1))
        cp = ctx.enter_context(tc.tile_pool(name="cp", bufs=1))
        bp = ctx.enter_context(tc.tile_pool(name="bp", bufs=1))
        gp = ctx.enter_context(tc.tile_pool(name="gp", bufs=8))
        zp = ctx.enter_context(tc.tile_pool(name="zp", bufs=2))
        wtp = ctx.enter_context(tc.tile_pool(name="wtp", bufs=3))
        tp = ctx.enter_context(tc.tile_pool(name="tp", bufs=2))
        pp = ctx.enter_context(tc.tile_pool(name="pp", bufs=8, space="PSUM"))

        bt = bp.tile([128, 8], F32, tag="bias")
        nc.sync.dma_start(bt[:], b_d[:])

        h16a = hp.tile([128, 2 * PLANE], BF16, tag="h16a")
        h16b = hp.tile([128, 2 * PLANE], BF16, tag="h16b")
        hf = hp.tile([128, 1024], F32, tag="hf")
        ct = cp.tile([128, 2 * CPL], F32, tag="c")
        nc.vector.memset(ct[:], 0.0)
        nc.vector.memset(h16a[:], 0.0)
        nc.vector.memset(h16b[:], 0.0)

        vx0 = vxp8.tile([128, 2 * 4 * VR8 * 16], F8, tag="vx8")
        nc.gpsimd.dma_start(vx0[:], vx8_d[0])

        # fp8 weight tiles, one per (dy, pos). x-conv i/o/g columns first
        # (t=0 skips f octiles and h-convs), issued in consumption order.
        u8x = [wp.tile([128, 2, 768], F8, tag=f"u8x{j}", name=f"u8x{j}")
               for j in range(12)]
        u8xf = [wp.tile([128, 2, 256], F8, tag=f"u8xf{j}", name=f"u8xf{j}")
                for j in range(12)]
        u8h = [wp.tile([128, 2, 1024], F8, tag=f"u8h{j}", name=f"u8h{j}")
               for j in range(12)]
        for j in range(12):
            nc.sync.dma_start(u8x[j][:], u8_d[j][:, :, :768])
        for j in range(12):
            nc.sync.dma_start(u8h[j][:], u8_d[12 + j])
        for j in range(12):
            nc.sync.dma_start(u8xf[j][:], u8_d[j][:, :, 768:])
        u16 = [wp.tile([128, 1024], BF16, tag=f"u16_{j}", name=f"u16_{j}")
               for j in range(48)]
        for j in range(48):
            nc.sync.dma_start(u16[j][:], u16_d[j])

        def u8slice(cv, dy, pos, o):
            j = dy * 4 + pos
            if cv == 0:
                if o < 6:
                    return u8x[j][:, :, o * 128:(o + 1) * 128]
                return u8xf[j][:, :, (o - 6) * 128:(o - 5) * 128]
            return u8h[j][:, :, o * 128:(o + 1) * 128]

        def u16slice(cv, dy, pos, it, o):
            j = ((cv * 3 + dy) * 4 + pos) * 2 + it
            return u16[j][:, o * 128:(o + 1) * 128]

        hbufs = [h16a, h16b]

        for t in range(T):
            fp8 = t < NF8
            r = 23 - t
            n, n2 = r * 32, r * 16
            VR = VR8 if fp8 else VR16
            if t == 0:
                vx = vx0
            elif fp8:
                vx = vxp8.tile([128, 2 * 4 * VR8 * 16], F8, tag="vx8")
                nc.gpsimd.dma_start(vx[:], vx8_d[t])
            else:
                vx = vxp16.tile([128, 2 * 4 * VR16 * 16], BF16, tag="vx16")
                nc.gpsimd.dma_start(vx[:], vx16_d[t - NF8])
            vxv = vx[:].rearrange("p (i s v j) -> p i s v j", i=2, s=4, v=VR, j=16)

            h_in = hbufs[t % 2]
            h_out = hbufs[(t + 1) % 2] if t < T - 1 else None

            # data transform for the h-conv: V = B^T h per 4-col window
            if t > 0:
                if fp8:
                    vh = vhp8.tile([128, 2 * 4 * VR8 * 16], F8, tag="vh8")
                else:
                    vh = vhp16.tile([128, 2 * 4 * VR16 * 16], BF16, tag="vh16")
                vhv = vh[:].rearrange("p (i s v j) -> p i s v j", i=2, s=4, v=VR, j=16)
                hw = h_in[:].rearrange("p (i v c two) -> p i v c two",
                                       i=2, v=ROWS, c=17, two=2)
                d0 = hw[:, :, 0:VR, 0:16, 0]
                d1 = hw[:, :, 0:VR, 0:16, 1]
                d2 = hw[:, :, 0:VR, 1:17, 0]
                d3 = hw[:, :, 0:VR, 1:17, 1]
                eng = nc.vector if fp8 else nc.gpsimd
                eng.tensor_sub(vhv[:, :, 0], d0, d2)
                eng.tensor_add(vhv[:, :, 1], d1, d2)
                eng.tensor_sub(vhv[:, :, 2], d2, d1)
                eng.tensor_sub(vhv[:, :, 3], d1, d3)

            hov = (h_out[:].rearrange("p (i v c) -> p i v c", i=2, v=ROWS, c=WC)
                   if h_out is not None else
                   hf[:].rearrange("p (i v c) -> p i v c", i=2, v=16, c=32))

            octs = [0, 1, 2, 3, 4, 5] if t == 0 else list(range(8))

            def x_mms(o, ps4):
                for dy in range(3):
                    for pos in range(4):
                        if fp8:
                            nc.tensor.matmul(
                                ps4[pos][:], u8slice(0, dy, pos, o),
                                vxv[:, :, pos, dy:dy + r, :],
                                start=(dy == 0), stop=(t == 0 and dy == 2),
                                perf_mode=DR, skip_group_check=True)
                        else:
                            for it in range(2):
                                nc.tensor.matmul(
                                    ps4[pos][:], u16slice(0, dy, pos, it, o),
                                    vxv[:, it, pos, dy:dy + r, :],
                                    start=(dy == 0 and it == 0), stop=False,
                                    skip_group_check=True)

            def h_mms(o, ps4):
                for dy in range(3):
                    for pos in range(4):
                        if fp8:
                            nc.tensor.matmul(
                                ps4[pos][:], u8slice(1, dy, pos, o),
                                vhv[:, :, pos, dy:dy + r, :],
                                start=False, stop=(dy == 2),
                                perf_mode=DR, skip_group_check=True)
                        else:
                            for it in range(2):
                                nc.tensor.matmul(
                                    ps4[pos][:], u16slice(1, dy, pos, it, o),
                                    vhv[:, it, pos, dy:dy + r, :],
                                    start=False, stop=(dy == 2 and it == 1),
                                    skip_group_check=True)

            gts = {}

            def drain(o, ps4):
                # z = A^T M: even cols m0+m1+m2, odd cols m1-m2-m3.
                # Only one PSUM operand per DVE op: stage m1 via ScalarE.
                zt = zp.tile([128, n], BF16, tag="z")
                zv = zt[:].rearrange("p (v j two) -> p v j two", v=r, j=16, two=2)
                s1 = wtp.tile([128, n2], BF16, tag="t01")
                t01 = wtp.tile([128, n2], BF16, tag="t01")
                t23 = wtp.tile([128, n2], BF16, tag="t01")
                m = [ps4[i][:].rearrange("p (v j) -> p v j", v=r, j=16)
                     for i in range(4)]
                s1v = s1[:].rearrange("p (v j) -> p v j", v=r, j=16)
                t01v = t01[:].rearrange("p (v j) -> p v j", v=r, j=16)
                t23v = t23[:].rearrange("p (v j) -> p v j", v=r, j=16)
                nc.scalar.activation(s1[:], ps4[1][:], AF.Copy)
                nc.vector.tensor_add(t01v, s1v, m[0])
                nc.vector.tensor_add(zv[:, :, :, 0], t01v, m[2])
                nc.vector.tensor_sub(t23v, s1v, m[2])
                nc.vector.tensor_sub(zv[:, :, :, 1], t23v, m[3])
                gt = gp.tile([128, n], BF16, tag="g")
                gts[o] = gt
                func = AF.Relu if o in (4, 5) else AF.Sigmoid
                nc.scalar.activation(gt[:], zt[:], func, bias=bt[:, o:o + 1],
                                     scale=DESCALE if fp8 else 1.0)

            def alloc4():
                return [pp.tile([128, n2], F32, tag="ps", name=f"ps{i}")
                        for i in range(4)]

            ps_map = {}
            ps_map[octs[0]] = alloc4()
            ps_map[octs[1]] = alloc4()
            x_mms(octs[0], ps_map[octs[0]])
            x_mms(octs[1], ps_map[octs[1]])
            for idx, o in enumerate(octs):
                if t > 0:
                    h_mms(o, ps_map[o])
                drain(o, ps_map[o])
                del ps_map[o]
                if idx + 2 < len(octs):
                    nxt = octs[idx + 2]
                    ps_map[nxt] = alloc4()
                    x_mms(nxt, ps_map[nxt])

            for hi in range(2):
                gi, go, gg = gts[0 + hi], gts[2 + hi], gts[4 + hi]
                c0 = hi * CPL
                cs = ct[:, c0: c0 + n]
                if t == 0:
                    nc.vector.tensor_mul(cs, gi[:], gg[:])
                else:
                    gf = gts[6 + hi]
                    nc.vector.tensor_mul(gg[:], gi[:], gg[:])
                    nc.vector.tensor_mul(cs, gf[:], cs)
                    nc.vector.tensor_add(cs, cs, gg[:])
                cr = tp.tile([128, n], BF16, tag="cr")
                if t < NF8 - 1:
                    # next step's conv consumes h in fp8 scaled by HS
                    nc.vector.tensor_scalar(cr[:], cs, 0.0, HS,
                                            ALU.max, ALU.mult)
                else:
                    nc.vector.tensor_scalar_max(cr[:], cs, 0.0)
                if t == T - 1:
                    nc.vector.tensor_mul(hov[:, hi, :, :], go[:], cr[:])
                else:
                    nc.vector.tensor_mul(hov[:, hi, 1: 1 + r, 1: 33],
                                         go[:], cr[:])

        for it in range(2):
            nc.sync.dma_start(out_d[it], hf[:].rearrange(
                "p (i v c) -> p i v c", i=2, v=16, c=32)[:, it, :, :])

    nc.compile()
    return nc


BT_W = np.array([[1, 0, -1, 0], [0, 1, 1, 0], [0, -1, 1, 0], [0, 1, 0, -1]],
                np.float32)
G_W = np.array([[1, 0, 0], [.5, .5, .5], [.5, -.5, .5], [0, 0, 1]], np.float32)

GATE_PERM = [0, 2, 3, 1]  # reorder [i, f, o, g] -> [i, o, g, f]


def _f8(a, scale):
    return np.clip(np.asarray(a, dtype=np.float32) * scale,
                   -240.0, 240.0).astype(ml_dtypes.float8_e4m3)


def _prep_weights(wx, wh, flip):
    ws = np.stack([np.asarray(wx), np.asarray(wh)])  # [2, 1024, 256, 3, 3]
    if flip:
        ws = ws[:, :, :, ::-1, :]
    # [cv, gate, ht, ch, it, ic, dy, dx]
    ws = ws.reshape(2, 4, 2, 128, 2, 128, 3, 3)[:, GATE_PERM]
    # U = G w over dx
    # u8: [cv, dy, pos, ic, it, (gate, ht, ch)]
    u8 = np.einsum('pa,cgemtida->cdpitgem', G_W, ws).reshape(2, 12, 128, 2, 1024)
    u8 = np.concatenate([_f8(u8[0], WXS), _f8(u8[1], WHS)])  # [24,128,2,1024]
    # u16: [cv, dy, pos, it, ic, (gate, ht, ch)]
    u16 = np.einsum('pa,cgemtida->cdptigem', G_W, ws).reshape(48, 128, 1024)
    return u8, u16.astype(ml_dtypes.bfloat16)


def _prep_x(xb, flip):
    # xb: [T, 256, 32, 32] for one batch element
    xc = np.asarray(xb, dtype=np.float32)
    if flip:
        xc = xc[:, :, ::-1, :]
    buf = np.zeros((T, 2, 128, ROWS, WC), dtype=np.float32)
    for it in range(2):
        buf[:, it, :, 1:25, 1:33] = xc[:, it * 128:(it + 1) * 128, 0:24, :]
    # V_x = B^T x over 4-col windows at stride 2
    cols = np.arange(16)[:, None] * 2 + np.arange(4)[None, :]
    xw = buf[:, :, :, :VR8, cols]            # [T, 2, 128, VR8, 16, 4]
    vx = np.einsum('pa,tiwvja->twipvj', BT_W, xw)  # [T, 128, 2, 4, VR8, 16]
    vx8 = _f8(vx[:NF8], XS).reshape(NF8, 128, 2 * 4 * VR8 * 16)
    vx16 = np.ascontiguousarray(vx[NF8:, :, :, :, :VR16, :]).astype(
        ml_dtypes.bfloat16).reshape(T - NF8, 128, 2 * 4 * VR16 * 16)
    return vx8, vx16


def kernel(x, wx, wh, bh):
    x = np.asarray(x, dtype=np.float32)
    B = x.shape[0]
    bias = np.ascontiguousarray(
        np.asarray(bh, dtype=np.float32).reshape(4, 2, 128)[GATE_PERM]
        .transpose(2, 0, 1).reshape(128, 8))

    u8_lo, u16_lo = _prep_weights(wx, wh, flip=False)
    u8_hi, u16_hi = _prep_weights(wx, wh, flip=True)

    in_maps = []
    for c in range(N_CORES):
        b, half = c // 2, c % 2
        vx8, vx16 = _prep_x(x[b], flip=bool(half))
        in_maps.append({
            "vx8": vx8,
            "vx16": vx16,
            "u8": u8_hi if half else u8_lo,
            "u16": u16_hi if half else u16_lo,
            "bias": bias,
        })

    if "nc" not in _cache:
        _cache["nc"] = _build_nc()
    nc = _cache["nc"]

    res = run_bass_kernel_spmd(nc, in_maps, core_ids=list(range(N_CORES)))
    _cache["last_results"] = res

    out = np.zeros((B, 256, 32, 32), dtype=np.float32)
    for c in range(N_CORES):
        b, half = c // 2, c % 2
        h = res.results[c]["hout"].reshape(2, 128, 16, 32)
        h = np.concatenate([h[0], h[1]], axis=0)  # [256, 16, 32]
        if half:
            out[b, :, 16:32, :] = h[:, ::-1, :]
        else:
            out[b, :, 0:16, :] = h
    return out
